# revision 111
# baseline (speedup 1.0000x reference)
"""Multi-head attention with relative-position-bias MLP on 8 TRN2 NeuronCores.

Strategy: pure data-parallel over batch (B=8 -> 1 batch element per core, no
collectives). Host-side prep is layout only: per-core transposed x (bf16),
transposed weights, replicated proj bias, and exp() of the 63x63
relative-position bias table. The token-reversed copy xRT is built ON-CHIP
from xT via DVE negative-stride copies (matmul operands cannot be
negative-stride, but DVE tensor_copy sources can).

Device algorithm per core (N=1024 tokens, C=768, H=12 heads, D=64):
  Pipelined per head-pair j: QKV matmuls for pair j+1 are interleaved into
  the attention inner loop of pair j, and scores run DEPTH=5 k-tiles ahead
  of the AV accumulation, so the PE array keeps busy while ACT computes
  exp(). All q/k/v tiles are bf16 (SBUF capacity, not matmul speed, is the
  binding constraint).

  qT[o,n] = wq[:,o].T @ xT        (bf16 ld+mm, f32 PSUM, bf16 SBUF copy)
  kT[o,n] = wk[:,o].T @ xRT       (token-reversed k)
  v[n,o]  = xRT.T @ wv            (token-reversed v -> vaug[tok, h*65+d] bf16
                                   with a ones column at h*65+64)
  per head h, k-tile kt (128 reversed tokens):
     sT = kT_h(kt).T @ qT_h       [128 k, 1024 q]  (bf16 matmul, PSUM)
     E  = exp(sT/8)               (ACT, bf16 out)
     P  = E * expB_view           (DVE, all-SBUF bf16; TBLREP trick:
                                   exp(s+b) = exp(s)*exp(b), b 2D-Toeplitz)
     av[h,ch] += vaug[kt][:, h-slice].T @ P[:, ch*512:+512]
                                  ([65, 512] PSUM; row 64 = denominator)
  division: DVE recip row 64 -> bf16 [1,512]; GPSIMD partition_broadcast
  -> rep[64,512] SBUF (deferred into the next head's iterations); DVE mul
  av(PSUM) * rep(SBUF) -> outT[c, n] f32r. Odd heads go via a tmp tile +
  partition-shift DMA; head 11 instead writes its own [64, N] tile ot11 so
  the projection's last input never waits on a DMA (the PSUM accumulation
  group waits at its FIRST matmul for ALL inputs).
  tail: proj y[n,o] = outT.T @ pwT (f32r self-loading) with the last
  c-block split into two K=64 steps (outT[5][0:64] + ot11 x pw5b); bias add
  on DVE, DMA out. Proj weights are DMA'd mid-kernel behind a 1-element WAW
  dep so the list scheduler cannot hoist them into the prologue's serial
  DMA window.

Token reversal trick: bias[h,n,m] depends on grid coords of (n,m) only via
(cy_n - cy_m, cx_n - cx_m). Reversing key/value token order makes the
Toeplitz expansion all-positive-stride: TBLREP_h[p, J] = expG_h[63*(p//32)
+ p%32 + J] (4 plain DMAs per head), and each [128,1024] bias tile is a
strided view of it. The AV reduction over k-tiles is order-invariant.
"""
import sys

import numpy as np

sys.path.insert(0, "/opt/trn_rl_repo")

import concourse.bass as bass  # noqa: E402
import concourse.mybir as mybir  # noqa: E402
import concourse.tile as tile  # noqa: E402
from concourse import bacc  # noqa: E402
from concourse.bass_utils import run_bass_kernel_spmd  # noqa: E402

F32 = mybir.dt.float32
F32R = mybir.dt.float32r
BF16 = mybir.dt.bfloat16
EXP = mybir.ActivationFunctionType.Exp

B, N, C, H, D = 8, 1024, 768, 12, 64
SCALE = float(D) ** -0.5
NT = N // 128   # 8 token tiles
CT = C // 128   # 6 channel tiles
NP = H // 2     # 6 head pairs
TBLW = 3781     # TBLREP width (padded so 2016-wide views stay in range)
TW = 4001       # DRAM table width per head (>= 220 + TBLW, zero-padded)


def _build_graph():
    nc = bacc.Bacc("TRN2", target_bir_lowering=False, debug=False,
                   enable_asserts=False, num_devices=B)
    xT_d = nc.dram_tensor("xT", [C, N], BF16, kind="ExternalInput")
    xRT_d = nc.dram_tensor("xRT", [C, N], BF16, kind="ExternalInput")
    wqkv_d = nc.dram_tensor("qkv_wT", [C, 3 * C], BF16, kind="ExternalInput")
    wproj_d = nc.dram_tensor("proj_wT", [C, C], F32, kind="ExternalInput")
    pbrep_d = nc.dram_tensor("proj_b_rep", [128, C], F32, kind="ExternalInput")
    tbl_d = nc.dram_tensor("rpb_tbl", [H, TW], BF16, kind="ExternalInput")
    out_d = nc.dram_tensor("out", [N, C], F32, kind="ExternalOutput")

    with tile.TileContext(nc) as tc:
        _kern(tc, nc, xT_d, xRT_d, wqkv_d, wproj_d, pbrep_d, tbl_d, out_d)
    nc.compile()
    return nc


def _kern(tc, nc, xT_d, xRT_d, wqkv_d, wproj_d, pbrep_d, tbl_d, out_d):
    from contextlib import ExitStack

    with ExitStack() as es:
        persist = es.enter_context(tc.tile_pool(name="persist", bufs=1))
        # q tiles 0..5, k tiles 6..11; [o-part(2 heads x 64), n-free] f32r
        qk_sb = [persist.tile([128, N], BF16, tag=f"qk{i}", name=f"qk{i}")
                 for i in range(12)]
        # v (token-reversed) head-strided with ones column at h*65+64
        vaug = [persist.tile([128, H * 65], BF16, tag=f"va{i}", name=f"va{i}")
                for i in range(NT)]
        # attention output transposed [c, n], c = h*64+d
        outT = [persist.tile([128, N], F32R, tag=f"ot{i}", name=f"ot{i}")
                for i in range(CT)]
        ot11 = persist.tile([64, N], BF16, tag="ot11")
        onescol = persist.tile([128, H], BF16, tag="onescol")
        nc.vector.memset(onescol[:], 1.0)
        for t in range(NT):
            va_v = vaug[t][:].rearrange("p (h e) -> p h e", e=65)
            nc.vector.tensor_copy(va_v[:, :, 64:65], onescol[:].unsqueeze(-1))

        ld = es.enter_context(tc.tile_pool(name="ld", bufs=1))
        xT = [ld.tile([128, N], BF16, tag=f"x{i}", name=f"x{i}")
              for i in range(CT)]
        xRT = [ld.tile([128, N], BF16, tag=f"xr{i}", name=f"xr{i}")
               for i in range(CT)]
        wq = [ld.tile([128, C], BF16, tag=f"wq{i}", name=f"wq{i}")
              for i in range(CT)]
        wk = [ld.tile([128, C], BF16, tag=f"wk{i}", name=f"wk{i}")
              for i in range(CT)]
        wv = [ld.tile([128, C], BF16, tag=f"wv{i}", name=f"wv{i}")
              for i in range(CT)]
        pwT = [ld.tile([128, C], F32R, tag=f"pw{i}", name=f"pw{i}")
               for i in range(CT)]
        pw5b = ld.tile([64, C], BF16, tag="pw5b")
        pbrow = ld.tile([128, C], F32, tag="pbrow")

        tblp = es.enter_context(tc.tile_pool(name="tblp", bufs=4))
        tbl_tiles = {}

        def tbl_dma(h, eng=None):
            eng = eng or nc.sync
            t = tblp.tile([128, TBLW], BF16, tag="tbl", name=f"tbl{h}")
            for blk in range(4):
                eng.dma_start(
                    t[blk * 32:(blk + 1) * 32, :],
                    bass.AP(tbl_d, h * TW + 63 * blk, [[1, 32], [1, TBLW]]))
            tbl_tiles[h] = t

        ep = es.enter_context(tc.tile_pool(name="expp", bufs=5))
        pp = es.enter_context(tc.tile_pool(name="phat", bufs=6))
        rcp = es.enter_context(tc.tile_pool(name="rcp", bufs=5))
        tmpp = es.enter_context(tc.tile_pool(name="tmpo", bufs=3))
        # PSUM: sc 2x4KB (4 banks) + qv 2x2KB (2) + av 2x2KB (2) = 8 banks
        scp = es.enter_context(tc.tile_pool(name="scp", bufs=2, space="PSUM"))
        qvp = es.enter_context(tc.tile_pool(name="qvp", bufs=2, space="PSUM"))
        avp = es.enter_context(tc.tile_pool(name="avp", bufs=2, space="PSUM"))

        # ---- qkv sub-chunk emitters (pair j): 24 half-chains each ----
        qkv_ps = {}

        def qkv_sub(j, idx):
            half, ci = divmod(idx, 12)  # ci: 0,1 q-halves; 2,3 k; 4..11 v
            first, last = half == 0, half == 1
            cts = range(0, 3) if first else range(3, CT)
            if ci < 4:
                ch = ci % 2
                dst = qk_sb[j] if ci < 2 else qk_sb[6 + j]
                wsrc = wq if ci < 2 else wk
                rhs_src = xT if ci < 2 else xRT
                if first:
                    qkv_ps[(j, ci)] = qvp.tile([128, 512], F32, tag="qv",
                                               name=f"qkps{j}_{ci}")
                ps = qkv_ps[(j, ci)]
                for ct in cts:
                    nc.tensor.matmul(
                        ps[:], wsrc[ct][:, j * 128:(j + 1) * 128],
                        rhs_src[ct][:, ch * 512:(ch + 1) * 512],
                        start=(ct == 0), stop=(ct == CT - 1))
                if last:
                    nc.vector.tensor_copy(dst[:, ch * 512:(ch + 1) * 512],
                                          ps[:])
            else:
                t = ci - 4
                if first:
                    qkv_ps[(j, ci)] = qvp.tile([128, 128], F32, tag="qv",
                                               name=f"vps{j}_{t}")
                ps = qkv_ps[(j, ci)]
                for ct in cts:
                    nc.tensor.matmul(
                        ps[:], xRT[ct][:, t * 128:(t + 1) * 128],
                        wv[ct][:, j * 128:(j + 1) * 128],
                        start=(ct == 0), stop=(ct == CT - 1))
                if last:
                    va_v = vaug[t][:].rearrange("p (h e) -> p h e", e=65)
                    ps_v = ps[:].rearrange("p (h d) -> p h d", d=64)
                    nc.vector.tensor_copy(va_v[:, 2 * j:2 * j + 2, 0:64],
                                          ps_v)

        def qkv_subs(j):
            for ci in (0, 1, 2, 3):
                yield (j, ci)
                yield (j, 12 + ci)
            for ci in range(4, 12):
                yield (j, ci)
                yield (j, 12 + ci)

        # prologue DMAs: tbl0 + q/k weights on the ACT hwdge queue,
        # x + v weights on the SP queue (two queues run concurrently)
        for i in range(3):
            nc.sync.dma_start(xT[i][:], xT_d.ap()[i * 128:(i + 1) * 128, :])
            nc.sync.dma_start(wq[i][:],
                              wqkv_d.ap()[i * 128:(i + 1) * 128, 0:C])
        tbl_dma(0)
        for i in range(3, CT):
            nc.sync.dma_start(xT[i][:], xT_d.ap()[i * 128:(i + 1) * 128, :])
            nc.sync.dma_start(wq[i][:],
                              wqkv_d.ap()[i * 128:(i + 1) * 128, 0:C])
        tbl_dma(1)
        for i in range(CT):
            # token-reversal on-chip: DVE copy with negative source stride
            nc.vector.tensor_copy(xRT[i][:], xT[i][:, ::-1])
            nc.sync.dma_start(wk[i][:],
                              wqkv_d.ap()[i * 128:(i + 1) * 128, C:2 * C])
        for i in range(CT):
            nc.sync.dma_start(wv[i][:],
                              wqkv_d.ap()[i * 128:(i + 1) * 128, 2 * C:3 * C])
        # full qkv for pair 0
        for jj, idx in qkv_subs(0):
            qkv_sub(jj, idx)

        # ---- main attention loop: scores pipelined 2 ahead of AV ----
        DEPTH = 5
        pending_div = []   # deferred rep+outmul ops from the previous head
        for j in range(NP):
            if j == 3:  # proj weights needed only at the tail; the WAW
                # dep on a pair-3 qkv output keeps the list scheduler from
                # hoisting these DMAs into the prologue's DMA window
                for i in range(CT):
                    nc.vector.tensor_copy(pwT[i][0:1, 0:1],
                                          qk_sb[3][0:1, 0:1])
                    nc.gpsimd.dma_start(
                        pwT[i][:], wproj_d.ap()[i * 128:(i + 1) * 128, :])
                nc.vector.tensor_copy(pw5b[0:1, 0:1], qk_sb[3][0:1, 0:1])
                nc.gpsimd.dma_start(pw5b[:], wproj_d.ap()[704:768, :])
                nc.sync.dma_start(pbrow[:], pbrep_d.ap()[:, :])
            filler = list(qkv_subs(j + 1)) if j < NP - 1 else []
            fptr = 0
            for hi in range(2):
                h = 2 * j + hi
                if h + 2 < H:
                    tbl_dma(h + 2)
                avs = [avp.tile([65, 512], F32, tag="av", name=f"av{h}_{c}")
                       for c in range(2)]
                phs = {}

                def stage_a(kt, h=h, hi=hi, j=j, phs=phs):
                    ps = scp.tile([128, 1024], F32, tag="sc",
                                  name=f"sc{h}_{kt}")
                    for ch in range(2):
                        nc.tensor.matmul(
                            ps[:, ch * 512:(ch + 1) * 512],
                            qk_sb[6 + j][hi * 64:hi * 64 + 64,
                                         kt * 128:(kt + 1) * 128],
                            qk_sb[j][hi * 64:hi * 64 + 64,
                                     ch * 512:(ch + 1) * 512],
                            start=True, stop=True)
                    ee = ep.tile([128, 1024], BF16, tag="ee",
                                 name=f"ee{h}_{kt}")
                    nc.scalar.activation(ee[:], ps[:], EXP, scale=SCALE)
                    tv = tbl_tiles[h][:, 252 * kt:252 * kt + 2016].rearrange(
                        "p (c a b) -> p c a b", c=2, b=63)[:, :, :, :32]
                    ph = pp.tile([128, 1024], BF16, tag="ph",
                                 name=f"ph{h}_{kt}")
                    pv = ph[:].rearrange("p (c a b) -> p c a b", c=2, b=32)
                    ev = ee[:].rearrange("p (c a b) -> p c a b", c=2, b=32)
                    nc.vector.tensor_mul(pv, ev, tv)
                    phs[kt] = ph

                def stage_b(kt, h=h, avs=avs, phs=phs):
                    ph = phs.pop(kt)
                    for ch in range(2):
                        nc.tensor.matmul(
                            avs[ch][:], vaug[kt][:, h * 65:(h + 1) * 65],
                            ph[:, ch * 512:(ch + 1) * 512],
                            start=(kt == 0), stop=(kt == NT - 1))

                for kt in range(NT + DEPTH):
                    i = hi * (NT + DEPTH) + kt
                    if kt < NT:
                        stage_a(kt)
                    # drain previous head's division (rep+outmul) early
                    if pending_div and kt < 6:
                        pending_div.pop(0)()
                    tgt = ((i + 1) * len(filler)) // (2 * (NT + DEPTH))
                    while fptr < tgt:
                        qkv_sub(*filler[fptr])
                        fptr += 1
                    if kt >= DEPTH:
                        stage_b(kt - DEPTH)
                # division part 1: reciprocals now (row 64 of av psums)
                rcs = []
                for ch in range(2):
                    rc = rcp.tile([128, 512], BF16, tag="rc",
                                   name=f"rc{h}_{ch}")
                    with nc.allow_low_precision(
                            reason="softmax recip rounded to f32r"):
                        nc.vector.reciprocal(rc[0:1, :], avs[ch][64:65, :])
                    rcs.append(rc)

                # division part 2: Pool broadcast (SBUF) + DVE mul, deferred
                def div_op(ch, h=h, hi=hi, j=j, avs=avs, rcs=rcs):
                    rep = rcp.tile([64, 512], BF16, tag="rep",
                                   name=f"rep{h}_{ch}", bufs=2)
                    nc.gpsimd.partition_broadcast(rep[:], rcs[ch][0:1, :],
                                                  channels=64)
                    if h == H - 1:
                        nc.vector.tensor_mul(
                            ot11[:, ch * 512:(ch + 1) * 512],
                            avs[ch][0:64, :], rep[:])
                        return
                    dst = outT[j][hi * 64:hi * 64 + 64,
                                  ch * 512:(ch + 1) * 512]
                    if hi == 0:
                        nc.vector.tensor_mul(dst, avs[ch][0:64, :], rep[:])
                    else:
                        tmp = tmpp.tile([64, 512], F32R, tag="tmo",
                                        name=f"tmo{h}_{ch}")
                        nc.vector.tensor_mul(tmp[:], avs[ch][0:64, :], rep[:])
                        nc.sync.dma_start(dst, tmp[:])
                for ch in range(2):
                    pending_div.append(lambda ch=ch, f=div_op: f(ch))
        for f in pending_div:
            f()

        # ---- tail: proj (psum via qv tag to stay in 8 banks) ----
        fsb = es.enter_context(tc.tile_pool(name="fsb", bufs=4))
        for qt in range(NT):
            for oc in range(2):
                f = fsb.tile([128, 384], F32, tag="f", name=f"f{qt}_{oc}")
                pj = qvp.tile([128, 384], F32, tag="qv", name=f"pj{qt}_{oc}")
                # ct 0..4 are ready during the last head's division; the
                # ct=5 half-steps come last so only they wait on head 11
                for ct in range(CT - 1):
                    nc.tensor.matmul(
                        pj[:], outT[ct][:, qt * 128:(qt + 1) * 128],
                        pwT[ct][:, oc * 384:(oc + 1) * 384],
                        start=(ct == 0), stop=False)
                nc.tensor.matmul(
                    pj[:], outT[5][0:64, qt * 128:(qt + 1) * 128],
                    pwT[5][0:64, oc * 384:(oc + 1) * 384],
                    start=False, stop=False)
                nc.tensor.matmul(
                    pj[:], ot11[:, qt * 128:(qt + 1) * 128],
                    pw5b[:, oc * 384:(oc + 1) * 384],
                    start=False, stop=True)
                nc.vector.tensor_add(f[:], pj[:],
                                     pbrow[:, oc * 384:(oc + 1) * 384])
                nc.sync.dma_start(
                    out_d.ap()[qt * 128:(qt + 1) * 128,
                               oc * 384:(oc + 1) * 384], f[:])


_GRAPH = None


def _graph():
    global _GRAPH
    if _GRAPH is None:
        _GRAPH = _build_graph()
    return _GRAPH


def _host_prep(x, qkv_w, proj_w, proj_b, rpb_w1, rpb_b1, rpb_w2, rpb_b2):
    """Numpy layout prep + exp of the 63x63 bias table (7 MFLOP)."""
    import ml_dtypes
    BFD = ml_dtypes.bfloat16
    a = np.arange(63, dtype=np.float32) - 31.0
    rel_y = np.broadcast_to(a[:, None], (63, 63))
    rel_x = np.broadcast_to(a[None, :], (63, 63))
    rel = np.stack([rel_x, rel_y], -1).reshape(-1, 2)           # [3969, 2]
    hdn = np.maximum(rel @ rpb_w1.T + rpb_b1, 0.0)
    gtbl = (hdn @ rpb_w2.T + rpb_b2).T.astype(np.float32)       # [12, 3969]
    gtbl = np.exp(gtbl, dtype=np.float32)                       # exp(bias)
    gpad = np.zeros((H, TW), np.float32)
    gpad[:, :3969] = gtbl
    gpad = gpad.astype(BFD)

    wqkvT = np.ascontiguousarray(qkv_w.T.astype(BFD))           # [768, 2304]
    wprojT = np.ascontiguousarray(proj_w.T.astype(np.float32))  # [768, 768]
    pbrep = np.ascontiguousarray(
        np.broadcast_to(proj_b.astype(np.float32), (128, C)))
    shared = {"qkv_wT": wqkvT, "proj_wT": wprojT, "proj_b_rep": pbrep,
              "rpb_tbl": gpad}
    in_maps = []
    for i in range(B):
        m = dict(shared)
        m["xT"] = np.ascontiguousarray(x[i].T.astype(BFD))
        m["xRT"] = np.ascontiguousarray(x[i][::-1].T.astype(BFD))
        in_maps.append(m)
    return in_maps


def kernel(x, qkv_w, proj_w, proj_b, rpb_w1, rpb_b1, rpb_w2, rpb_b2,
           _trace=False, _tmpdir=None):
    in_maps = _host_prep(np.asarray(x), np.asarray(qkv_w), np.asarray(proj_w),
                         np.asarray(proj_b), np.asarray(rpb_w1),
                         np.asarray(rpb_b1), np.asarray(rpb_w2),
                         np.asarray(rpb_b2))
    nc = _graph()
    res = run_bass_kernel_spmd(nc, in_maps, core_ids=list(range(B)),
                               trace=_trace, tmpdir=_tmpdir)
    out = np.stack([res.results[i]["out"] for i in range(B)])
    if _trace:
        kernel._last_results = res
    return out
